# revision 50
# baseline (speedup 1.0000x reference)
"""Trainium2 Bass kernel for nn_PixtralHFVisionModel (8-core TP).

Strategy (Megatron tensor-parallel over 8 NeuronCores, one chip):
  - Patch-embed conv done as matmul (host im2col), replicated on all cores.
  - Activations live TRANSPOSED in SBUF: [hidden(partitions, 8x128), S(free)].
  - Per-core: 2 of 16 heads (q/k/v/o column/row split), 512 of 4096 MLP
    intermediate channels.
  - The sequence is processed as three uniform 512-token chunks
    (C0,C1 = img0, C2 = img1).  o-proj / down-proj partial sums are
    AllReduced per *chunk* (1 MB bf16 each, 6 per layer) so every AR
    hides behind the other chunks' compute; each core keeps the full
    f32 residual stream.
  - Attention per image block (block-diagonal mask is free).  Scores are
    built transposed [kv, q] so softmax-exp output feeds the AV matmul
    directly; denominator via an appended ones-column in the V operand.
  - exp without max subtraction (scores*scale is provably small here).
  - RoPE: rotate-half done as partition-swap DMAs + signed sin table.
  - RMSNorm weights (attn_norm/ffn_norm) are folded into the q/k/v and
    gate/up weights on the host, so the norm apply is a single
    tensor-tensor multiply by the broadcast rstd.
All matmuls bf16 inputs, f32 PSUM accumulation.
"""
import sys

if "/opt/trn_rl_repo" not in sys.path:
    sys.path.insert(0, "/opt/trn_rl_repo")

import numpy as np
import ml_dtypes

BF16 = ml_dtypes.bfloat16
NCORES = 8
HID = 1024
HD = 64
INTER = 4096
NLAYERS = 4
PATCH = 16
MAXSIDE = 64
THETA = 10000.0
EPS = 1e-5
SCALE = HD ** -0.5
GRIDS = [(32, 32), (32, 16)]
S0, S1 = 1024, 512
S = S0 + S1
CH = 512            # tokens per chunk == free-dim matmul size (1 PSUM bank)
KT = HID // 128     # 8 hidden k-tiles
PKT = 768 // 128    # 6 patch k-tiles
MT_I = 512 // 128   # 4 intermediate m-tiles per core
NCHUNK = 3
C_IMG = [0, 0, 1]          # chunk -> image
C_LO = [0, 512, 1024]      # chunk -> global token offset
C_LOC = [0, 512, 0]        # chunk -> offset inside its image
IMG_CHUNKS = [[0, 1], [2]]
NKV = [8, 4]               # kv 128-tiles per image
IMW = [1024, 512]

_CACHE = {}


def _build_nc():
    import concourse.bacc as bacc
    from concourse import tile
    import concourse.mybir as mybir

    dt = mybir.dt
    f32, bf16 = dt.float32, dt.bfloat16
    AF = mybir.ActivationFunctionType
    ALU = mybir.AluOpType

    nc = bacc.Bacc("TRN2", target_bir_lowering=False, debug=False,
                   num_devices=NCORES)
    marks = []

    def mark(label):
        marks.append((label, nc.next_id()))

    def din(name, shape, dtype=bf16):
        return nc.dram_tensor(name, shape, dtype, kind="ExternalInput")

    patchesT_d = din("patchesT", [128, PKT, S])
    convWT_d = din("convWT", [128, PKT, HID])
    eye8_d = din("eye8", [128, 128])
    cos2_d = din("cos2", [128, S])
    sin2_d = din("sin2", [128, S])
    lnw_d = din("lnw", [128, KT], f32)
    wq_d = din("wq", [NLAYERS, 128, KT, 128])
    wk_d = din("wk", [NLAYERS, 128, KT, 128])
    wv2_d = din("wv2", [NLAYERS, 128, KT, 130])
    wo0_d = din("wo0", [NLAYERS, 64, KT, 128])
    wo1_d = din("wo1", [NLAYERS, 64, KT, 128])
    wg_d = din("wg", [NLAYERS, 128, KT, 512])
    wu_d = din("wu", [NLAYERS, 128, KT, 512])
    wd_d = din("wd", [NLAYERS, 128, MT_I, HID])
    out_d = nc.dram_tensor("out", [128, KT, S], f32, kind="ExternalOutput")

    import concourse.bass as bass_mod

    with tile.TileContext(nc) as tc:
        with (
            tc.tile_pool(name="const", bufs=1) as constp,
            tc.tile_pool(name="big", bufs=1) as bigp,
            tc.tile_pool(name="wat", bufs=1) as watp,
            tc.tile_pool(name="wmlp", bufs=1) as wmlpp,
            tc.tile_pool(name="att", bufs=1) as attp,
            tc.tile_pool(name="wrk1", bufs=2) as wrk1p,
            tc.tile_pool(name="wrk2", bufs=2) as wrk2p,
            tc.tile_pool(name="cast", bufs=2) as castp,
            tc.tile_pool(name="stage", bufs=2) as stagep,
            tc.tile_pool(name="dram", bufs=2, space="DRAM") as dramp,
            tc.tile_pool(name="psA", bufs=3, space="PSUM") as psA,
            tc.tile_pool(name="psB", bufs=2, space="PSUM") as psB,
            tc.tile_pool(name="psC", bufs=2, space="PSUM") as psC,
            tc.tile_pool(name="psS", bufs=1, space="PSUM") as psS,
        ):
            # ---- persistent tiles ----
            cos2 = constp.tile([128, S], bf16, tag="cos2")
            sin2 = constp.tile([128, S], bf16, tag="sin2")
            ones1 = constp.tile([128, 1], bf16, tag="ones1")
            onesb = constp.tile([65, 128], f32, tag="onesb")
            onesh = constp.tile([1, 128], bf16, tag="onesh")
            epsc = constp.tile([128, 1], f32, tag="epsc")
            lnw = constp.tile([128, KT], f32, tag="lnw")
            eye8 = constp.tile([128, 128], bf16, tag="eye8")
            nc.sync.dma_start(cos2[:], cos2_d[:])
            nc.sync.dma_start(sin2[:], sin2_d[:])
            nc.sync.dma_start(lnw[:], lnw_d[:])
            nc.sync.dma_start(eye8[:], eye8_d[:])
            nc.gpsimd.memset(ones1[:], 1.0)
            nc.gpsimd.memset(onesb[:], 1.0)
            nc.gpsimd.memset(onesh[:], 1.0)
            nc.gpsimd.memset(epsc[:], EPS)

            def act_raw(out, in_, func, bias=0.0, scale=1.0):
                """activation() without the Rsqrt/Reciprocal accuracy guard."""
                eng = nc.scalar
                inputs = [eng.lower_ap(in_)]
                for arg in (bias, scale, 0.0):
                    if isinstance(arg, bass_mod.AP):
                        inputs.append(eng.lower_ap(arg))
                    else:
                        inputs.append(mybir.ImmediateValue(
                            dtype=f32, value=float(arg)))
                return eng.add_instruction(mybir.InstActivation(
                    name=f"I-{nc.next_id()}", func=func,
                    ins=inputs, outs=[eng.lower_ap(out)]))

            # per-chunk residual: the current AllReduce output tile
            # [128, KT*CH] bf16 (ring-allocated).  resid/8 is re-injected
            # into every o/down projection via an eye/8 matmul so the AR
            # output IS the new residual (no post-AR adds).
            rarr = [None] * NCHUNK

            def new_rarr(c):
                t = stagep.tile([128, KT * CH], bf16, tag="arr", bufs=5,
                                name=f"arr{c}")
                rarr[c] = t
                return t

            def rsl(kt):
                return slice(kt * CH, (kt + 1) * CH)

            xnorms = [[bigp.tile([128, CH], bf16, tag=f"xn{c}_{k}",
                                 name=f"xn{c}_{k}") for k in range(KT)]
                      for c in range(NCHUNK)]
            hmlps = [[bigp.tile([128, CH], bf16, tag=f"hm{c}_{m}",
                                name=f"hm{c}_{m}") for m in range(MT_I)]
                     for c in range(NCHUNK)]
            # per-image K (rope'd) and V2 (kv-major with ones columns)
            kts = [attp.tile([128, IMW[i], ], bf16, tag=f"kt{i}",
                             name=f"kts{i}") for i in range(2)]
            v2s = [attp.tile([128, NKV[i], 130], bf16, tag=f"v2{i}",
                             name=f"v2s{i}") for i in range(2)]
            qts = [attp.tile([128, CH], bf16, tag=f"qt{c}",
                             name=f"qts{c}") for c in range(NCHUNK)]
            for i in range(2):
                nc.gpsimd.memset(v2s[i][:, :, 64:65], 1.0)
                nc.gpsimd.memset(v2s[i][:, :, 129:130], 1.0)

            def rms_rstd(c):
                """rstdb [128,CH] f32 PSUM = bcast(rsqrt(mean(resid^2)+eps)).

                Squares alternate vector/gpsimd so they pipeline; the
                partition-broadcast of rstd is a ones outer-product matmul
                (avoids a gpsimd hop on the post-AR latency chain)."""
                src = rarr[c]
                pss = psS.tile([128, CH], f32, tag="pss")
                for kt in range(KT):
                    sq = castp.tile([128, CH], bf16, tag="sq")
                    eng = nc.vector if kt % 2 == 0 else nc.gpsimd
                    eng.tensor_mul(sq[:], src[:, rsl(kt)], src[:, rsl(kt)])
                    nc.tensor.matmul(pss[0:1, :], lhsT=ones1[:], rhs=sq[:],
                                     start=(kt == 0), stop=(kt == KT - 1))
                rstd0 = wrk1p.tile([1, CH], bf16, tag="rstd0", bufs=1)
                act_raw(rstd0[:], pss[0:1, :], AF.Rsqrt,
                        bias=epsc[0:1, :], scale=1.0 / HID)
                nc.tensor.matmul(pss[:], lhsT=onesh[:], rhs=rstd0[:],
                                 start=True, stop=True)
                return pss

            def norm(c):
                """xnorm_c = bf16(resid_c * rstd)  (norm wt folded into W)."""
                rstdb = rms_rstd(c)
                src = rarr[c]
                for kt in range(KT):
                    nc.vector.tensor_mul(xnorms[c][kt][:], src[:, rsl(kt)],
                                         rstdb[:])

            def ln_pre(c):
                """resid_c = bf16(resid_c * lnw * rstd)  (once, at start)."""
                rstdb = rms_rstd(c)
                src = rarr[c]
                dst = new_rarr(c)
                for kt in range(KT):
                    nc.vector.scalar_tensor_tensor(
                        dst[:, rsl(kt)], src[:, rsl(kt)], lnw[:, kt:kt + 1],
                        rstdb[:], ALU.mult, ALU.mult)

            def readback(c, arout):
                """new resid_c = AllReduce output (two split DMAs)."""
                arr = new_rarr(c)
                H = KT * CH // 2
                nc.sync.dma_start(arr[:, 0:H], arout[:, 0:H])
                nc.sync.dma_start(arr[:, H:], arout[:, H:])

            def rope(dst, lo):
                """in-place rope on dst [128, CH] at global offset lo."""
                rot = wrk1p.tile([128, CH], bf16, tag="rot")
                t1 = wrk1p.tile([128, CH], bf16, tag="t1")
                for h in range(2):
                    b = h * 64
                    nc.sync.dma_start(rot[b:b + 32, :], dst[b + 32:b + 64, :])
                    nc.sync.dma_start(rot[b + 32:b + 64, :], dst[b:b + 32, :])
                nc.vector.tensor_mul(t1[:], dst[:], cos2[:, lo:lo + CH])
                nc.vector.tensor_mul(rot[:], rot[:], sin2[:, lo:lo + CH])
                nc.vector.tensor_add(dst[:], t1[:], rot[:])

            def k_proj(c, wk):
                img, loc, lo = C_IMG[c], C_LOC[c], C_LO[c]
                psq = psA.tile([128, CH], f32, tag="psx")
                for kt in range(KT):
                    nc.tensor.matmul(psq[:], lhsT=wk[:, kt, :],
                                     rhs=xnorms[c][kt][:],
                                     start=(kt == 0), stop=(kt == KT - 1))
                dst = kts[img][:, loc:loc + CH]
                nc.scalar.activation(dst, psq[:], AF.Copy)
                rope(dst, lo)

            def q_proj(c, wq):
                psq = psA.tile([128, CH], f32, tag="psx")
                for kt in range(KT):
                    nc.tensor.matmul(psq[:], lhsT=wq[:, kt, :],
                                     rhs=xnorms[c][kt][:],
                                     start=(kt == 0), stop=(kt == KT - 1))
                nc.scalar.activation(qts[c][:], psq[:], AF.Copy)
                rope(qts[c], C_LO[c])

            def v_proj(img, wv2):
                # ones columns 64/129 were preset once at startup; copy only
                # the V halves so no per-layer memset gates the AV matmuls
                for kv in range(NKV[img]):
                    c = IMG_CHUNKS[img][kv // 4]
                    loc = (kv % 4) * 128
                    psv = psB.tile([128, 512], f32, tag="psb")
                    for kt in range(KT):
                        nc.tensor.matmul(
                            psv[:, 0:130],
                            lhsT=xnorms[c][kt][:, loc:loc + 128],
                            rhs=wv2[:, kt, :],
                            start=(kt == 0), stop=(kt == KT - 1))
                    nc.scalar.activation(v2s[img][:, kv, 0:64], psv[:, 0:64],
                                         AF.Copy)
                    nc.scalar.activation(v2s[img][:, kv, 65:129],
                                         psv[:, 65:129], AF.Copy)

            def attn(c, wo0, wo1):
                """attention + o-proj for one chunk -> AR input dram tile."""
                img = C_IMG[c]
                kt_t, v2, qt = kts[img], v2s[img], qts[c]
                hsls = [slice(h * 64, (h + 1) * 64) for h in range(2)]
                vsls = [slice(h * 65, h * 65 + 65) for h in range(2)]
                # both heads interleaved as paired streams so each AV's exp
                # finishes during the other head's scores matmul
                psavs = [psC.tile([65, CH], f32, tag="psav",
                                  name=f"psav{h}") for h in range(2)]
                for i in range(NKV[img]):
                    pts = []
                    for h in range(2):
                        pss = psB.tile([128, 512], f32, tag="psb")
                        nc.tensor.matmul(
                            pss[:, 0:CH],
                            lhsT=kt_t[hsls[h], i * 128:(i + 1) * 128],
                            rhs=qt[hsls[h], :],
                            start=True, stop=True)
                        pt = castp.tile([128, CH], bf16, tag="pt",
                                        bufs=4, name=f"pt{h}")
                        nc.scalar.activation(pt[:], pss[:, 0:CH],
                                             AF.Exp, scale=SCALE)
                        pts.append(pt)
                    for h in range(2):
                        nc.tensor.matmul(
                            psavs[h][:], lhsT=v2[:, i, vsls[h]],
                            rhs=pts[h][:],
                            start=(i == 0), stop=(i == NKV[img] - 1))
                ots = []
                for h in range(2):
                    rec = wrk2p.tile([65, CH], f32, tag="rec")
                    nc.vector.reciprocal(rec[64:65, :], psavs[h][64:65, :])
                    avs = wrk2p.tile([64, CH], bf16, tag=f"avs{h}",
                                     name=f"avs{h}")
                    nc.scalar.activation(avs[:], psavs[h][0:64, :], AF.Copy)
                    ob = psB.tile([128, 512], f32, tag="psb")
                    nc.tensor.matmul(ob[0:64, 0:CH],
                                     lhsT=onesb[64:65, 0:64],
                                     rhs=rec[64:65, :],
                                     start=True, stop=True)
                    ot = wrk2p.tile([64, CH], bf16, tag=f"ot{h}",
                                    name=f"ot{h}")
                    nc.vector.tensor_mul(ot[:], avs[:], ob[0:64, 0:CH])
                    ots.append(ot)
                # o-projection + resid/8 -> bf16 -> per-chunk AllReduce
                dsm = stagep.tile([128, KT * CH], bf16, tag="dsm",
                                  name=f"dsa{c}")
                res = rarr[c]
                for kt in range(KT):
                    pso = psA.tile([128, CH], f32, tag="psx")
                    nc.tensor.matmul(pso[:], lhsT=wo0[:, kt, :], rhs=ots[0][:],
                                     start=True, stop=False)
                    nc.tensor.matmul(pso[:], lhsT=wo1[:, kt, :], rhs=ots[1][:],
                                     start=False, stop=False)
                    nc.tensor.matmul(pso[:], lhsT=eye8[:],
                                     rhs=res[:, rsl(kt)],
                                     start=False, stop=True)
                    nc.vector.tensor_copy(dsm[:, rsl(kt)], pso[:])
                return start_ar(c, "a", dsm)

            def start_ar(c, tagc, dsm):
                arin = dramp.tile([128, KT * CH], bf16, tag=f"ari{tagc}{c}",
                                  name=f"ari{tagc}{c}")
                arout = dramp.tile([128, KT * CH], bf16, tag=f"aro{tagc}{c}",
                                   name=f"aro{tagc}{c}", addr_space="Shared")
                nc.sync.dma_start(arin[:], dsm[:])
                nc.gpsimd.collective_compute(
                    "AllReduce", ALU.add,
                    ins=[arin.opt()], outs=[arout.opt()],
                    replica_groups=[list(range(NCORES))])
                return arout

            def mlp(c, wg, wu, wd):
                for mt in range(MT_I):
                    msl = slice(mt * 128, (mt + 1) * 128)
                    psg = psA.tile([128, CH], f32, tag="psx")
                    for kt in range(KT):
                        nc.tensor.matmul(
                            psg[:], lhsT=wg[:, kt, msl], rhs=xnorms[c][kt][:],
                            start=(kt == 0), stop=(kt == KT - 1))
                    gts = castp.tile([128, CH], bf16, tag="gts", bufs=2)
                    nc.scalar.activation(gts[:], psg[:], AF.Silu)
                    psu = psB.tile([128, 512], f32, tag="psb")
                    for kt in range(KT):
                        nc.tensor.matmul(
                            psu[:, 0:CH], lhsT=wu[:, kt, msl],
                            rhs=xnorms[c][kt][:],
                            start=(kt == 0), stop=(kt == KT - 1))
                    nc.vector.tensor_mul(hmlps[c][mt][:], gts[:],
                                         psu[:, 0:CH])
                dsm = stagep.tile([128, KT * CH], bf16, tag="dsm",
                                  name=f"dsm{c}")
                res = rarr[c]
                for kt in range(KT):
                    psd = psA.tile([128, CH], f32, tag="psx")
                    for mt in range(MT_I):
                        nc.tensor.matmul(
                            psd[:], lhsT=wd[:, mt, kt * 128:(kt + 1) * 128],
                            rhs=hmlps[c][mt][:],
                            start=(mt == 0), stop=False)
                    nc.tensor.matmul(psd[:], lhsT=eye8[:],
                                     rhs=res[:, rsl(kt)],
                                     start=False, stop=True)
                    if kt % 2 == 0:
                        nc.scalar.activation(dsm[:, rsl(kt)], psd[:], AF.Copy)
                    else:
                        nc.vector.tensor_copy(dsm[:, rsl(kt)], psd[:])
                return start_ar(c, "m", dsm)

            # ---- conv patch embed (replicated, streamed) + ln_pre ----
            with tc.tile_pool(name="convp", bufs=2) as convp:
                for c in range(NCHUNK):
                    gco = C_LO[c]
                    pch = convp.tile([128, PKT, CH], bf16, tag="pch")
                    nc.sync.dma_start(pch[:], patchesT_d[:, :, gco:gco + CH])
                    cr = new_rarr(c)
                    for kt in range(KT):
                        cwt = convp.tile([128, PKT, 128], bf16, tag="cwt")
                        nc.sync.dma_start(
                            cwt[:], convWT_d[:, :, kt * 128:(kt + 1) * 128])
                        psx = psA.tile([128, CH], f32, tag="psx")
                        for pk in range(PKT):
                            nc.tensor.matmul(
                                psx[:], lhsT=cwt[:, pk, :],
                                rhs=pch[:, pk, :],
                                start=(pk == 0), stop=(pk == PKT - 1))
                        nc.scalar.activation(cr[:, rsl(kt)], psx[:], AF.Copy)
                    ln_pre(c)

            # ---- transformer layers (3-chunk pipelined stream) ----
            ar_m = [None, None, None]
            for l in range(NLAYERS):
                wq = watp.tile([128, KT, 128], bf16, tag="wq")
                wk = watp.tile([128, KT, 128], bf16, tag="wk")
                wv2 = watp.tile([128, KT, 130], bf16, tag="wv2")
                wo0 = watp.tile([64, KT, 128], bf16, tag="wo0")
                wo1 = watp.tile([64, KT, 128], bf16, tag="wo1")
                nc.sync.dma_start(wq[:], wq_d[l])
                nc.sync.dma_start(wk[:], wk_d[l])
                nc.sync.dma_start(wv2[:], wv2_d[l])
                nc.sync.dma_start(wo0[:], wo0_d[l])
                nc.sync.dma_start(wo1[:], wo1_d[l])
                wg = wmlpp.tile([128, KT, 512], bf16, tag="wg")
                wu = wmlpp.tile([128, KT, 512], bf16, tag="wu")
                wd = wmlpp.tile([128, MT_I, HID], bf16, tag="wd")
                nc.sync.dma_start(wg[:], wg_d[l])
                nc.sync.dma_start(wu[:], wu_d[l])
                nc.sync.dma_start(wd[:], wd_d[l])

                # attn norms + q/k for chunks 0,1 first so attention on img0
                # can start while chunk 2's AR from the previous layer lands
                # readbacks are emitted one chunk ahead of their consumer so
                # they never sit behind a later arin-write in the sync queue
                if ar_m[2] is not None:
                    readback(2, ar_m[2])
                for c in (0, 1):
                    mark(f"L{l}nA{c}")
                    norm(c)
                    mark(f"L{l}kq{c}")
                    k_proj(c, wk)
                    q_proj(c, wq)
                mark(f"L{l}v0")
                v_proj(0, wv2)
                mark(f"L{l}nA2")
                norm(2)
                mark(f"L{l}kq2")
                k_proj(2, wk)
                q_proj(2, wq)
                mark(f"L{l}v1")
                v_proj(1, wv2)
                ar_a = []
                for c in range(NCHUNK):
                    mark(f"L{l}at{c}")
                    ar_a.append(attn(c, wo0, wo1))
                    if c >= 1:
                        readback(c - 1, ar_a[c - 1])
                for c in range(NCHUNK):
                    mark(f"L{l}nF{c}")
                    norm(c)
                    mark(f"L{l}ml{c}")
                    if c >= 1:
                        readback(c - 1, ar_m[c - 1])
                    ar_m[c] = mlp(c, wg, wu, wd)
                    if c == 0:
                        readback(2, ar_a[2])

            def wout(c):
                lo = C_LO[c]
                for kt in range(KT):
                    fin = wrk1p.tile([128, CH], f32, tag="fin", bufs=2)
                    nc.scalar.activation(fin[:], rarr[c][:, rsl(kt)], AF.Copy)
                    nc.sync.dma_start(out_d[:, kt, lo:lo + CH], fin[:])

            wout(0)
            wout(1)
            readback(2, ar_m[2])
            wout(2)

    mark("end")
    nc.compile()
    import json as _json
    with open("/tmp/phase_marks.json", "w") as f:
        _json.dump(marks, f)
    return nc


# ---------------- host-side prep ----------------

def _im2col(img):
    C, H, W = img.shape
    h, w = H // PATCH, W // PATCH
    p = img.reshape(C, h, PATCH, w, PATCH).transpose(1, 3, 0, 2, 4)
    return p.reshape(h * w, C * PATCH * PATCH)


def _rope_tables():
    freqs = 1.0 / THETA ** (np.arange(0, HD, 2, dtype=np.float64) / HD)
    fh = np.outer(np.arange(MAXSIDE, dtype=np.float64), freqs[::2])
    fw = np.outer(np.arange(MAXSIDE, dtype=np.float64), freqs[1::2])
    pids = np.concatenate([
        (np.arange(h)[:, None] * MAXSIDE + np.arange(w)[None, :]).reshape(-1)
        for h, w in GRIDS])
    inv = np.concatenate([
        np.broadcast_to(fh[:, None, :], (MAXSIDE, MAXSIDE, HD // 4)),
        np.broadcast_to(fw[None, :, :], (MAXSIDE, MAXSIDE, HD // 4))],
        axis=-1).reshape(-1, HD // 2)
    inv = np.concatenate([inv, inv], axis=-1)
    emb = inv[pids]                                   # [S, 64]
    cosT = np.cos(emb).T.astype(np.float32)           # [64, S]
    sinT = np.sin(emb).T.astype(np.float32)
    sinTs = np.concatenate([-sinT[:32], sinT[32:]], axis=0)
    cos2 = np.concatenate([cosT, cosT], axis=0).astype(BF16)
    sin2 = np.concatenate([sinTs, sinTs], axis=0).astype(BF16)
    return np.ascontiguousarray(cos2), np.ascontiguousarray(sin2)


def _ktile(a, last):
    """[L, 1024, last] -> [L, 128, kt, last] (partition-major k-tiles)."""
    L = a.shape[0]
    return np.ascontiguousarray(
        a.reshape(L, -1, 128, last).transpose(0, 2, 1, 3))


def _prep(inputs):
    f32 = np.float32
    patches = np.concatenate([
        _im2col(np.asarray(inputs["img0"], f32)),
        _im2col(np.asarray(inputs["img1"], f32))])          # [S, 768]
    patchesT = np.ascontiguousarray(
        patches.T.reshape(PKT, 128, S).transpose(1, 0, 2)).astype(BF16)
    cw = np.asarray(inputs["conv_w"], f32).reshape(HID, 768)
    convWT = np.ascontiguousarray(
        cw.T.reshape(PKT, 128, HID).transpose(1, 0, 2)).astype(BF16)
    cos2, sin2 = _rope_tables()
    lnw = np.ascontiguousarray(
        np.asarray(inputs["ln_pre_w"], f32).reshape(KT, 128).T)

    anw = np.asarray(inputs["attn_norm_w"], f32)              # [4, 1024]
    fnw = np.asarray(inputs["ffn_norm_w"], f32)
    # fold the norm weights into the following projections (input dim)
    qwT = np.asarray(inputs["q_w"], f32).transpose(0, 2, 1) * anw[:, :, None]
    kwT = np.asarray(inputs["k_w"], f32).transpose(0, 2, 1) * anw[:, :, None]
    vwT = np.asarray(inputs["v_w"], f32).transpose(0, 2, 1) * anw[:, :, None]
    owT = np.asarray(inputs["o_w"], f32).transpose(0, 2, 1)   # [4, d, e]
    gwT = np.asarray(inputs["gate_w"], f32).transpose(0, 2, 1) * fnw[:, :, None]
    uwT = np.asarray(inputs["up_w"], f32).transpose(0, 2, 1) * fnw[:, :, None]
    dwT = np.asarray(inputs["down_w"], f32).transpose(0, 2, 1)  # [4, I, out]

    eye8 = (np.eye(128, dtype=f32) * 0.125).astype(BF16)
    common = dict(patchesT=patchesT, convWT=convWT, cos2=cos2, sin2=sin2,
                  lnw=lnw, eye8=eye8)
    in_maps = []
    for c in range(NCORES):
        esl = slice(c * 128, (c + 1) * 128)
        isl = slice(c * 512, (c + 1) * 512)
        wv = vwT[:, :, esl].astype(BF16)                      # [4, 1024, 128]
        wv2 = np.zeros((NLAYERS, HID, 130), BF16)
        wv2[:, :, 0:64] = wv[:, :, 0:64]
        wv2[:, :, 65:129] = wv[:, :, 64:128]
        wo = owT[:, esl, :]                                   # [4, 128, 1024]
        m = dict(
            wq=_ktile(qwT[:, :, esl].astype(BF16), 128),
            wk=_ktile(kwT[:, :, esl].astype(BF16), 128),
            wv2=_ktile(wv2, 130),
            wo0=np.ascontiguousarray(
                wo[:, 0:64, :].reshape(NLAYERS, 64, KT, 128)).astype(BF16),
            wo1=np.ascontiguousarray(
                wo[:, 64:128, :].reshape(NLAYERS, 64, KT, 128)).astype(BF16),
            wg=_ktile(gwT[:, :, isl].astype(BF16), 512),
            wu=_ktile(uwT[:, :, isl].astype(BF16), 512),
            wd=np.ascontiguousarray(
                dwT[:, isl, :].reshape(NLAYERS, MT_I, 128, HID)
                .transpose(0, 2, 1, 3)).astype(BF16),
            **common)
        in_maps.append(m)
    return in_maps


LAST_RESULTS = None
TRACE = False


def _install_ntff_hook():
    """The RL container's antenv lacks axon_hooks; recreate it so
    trace=True can capture NTFF profiles through the axon terminal."""
    import types
    import antenv

    if hasattr(antenv, "axon_hooks"):
        return
    mod = types.ModuleType("antenv.axon_hooks")
    holder = [None]
    mod.set_axon_ntff_profile_hook = lambda h: holder.__setitem__(0, h)
    mod.get_axon_ntff_profile_hook = lambda: holder[0]
    sys.modules["antenv.axon_hooks"] = mod
    antenv.axon_hooks = mod
    if "/root/.axon_site" not in sys.path:
        sys.path.insert(0, "/root/.axon_site")
    try:
        from trn_agent_boot.trn_boot import _ntff_profile_via_ctypes
        mod.set_axon_ntff_profile_hook(
            _ntff_profile_via_ctypes("/opt/axon/libaxon_pjrt.so"))
    except Exception as e:  # pragma: no cover
        print("ntff hook install failed:", e)


def kernel(**inputs):
    global LAST_RESULTS
    from concourse import bass_utils

    if TRACE:
        _install_ntff_hook()

    if "nc" not in _CACHE:
        _CACHE["nc"] = _build_nc()
    nc = _CACHE["nc"]
    in_maps = _prep(inputs)
    res = bass_utils.run_bass_kernel_spmd(
        nc, in_maps, core_ids=list(range(NCORES)), trace=TRACE)
    LAST_RESULTS = res
    out = res.results[0]["out"]                 # [128, KT, S] f32
    full = out.transpose(1, 0, 2).reshape(HID, S)   # [hid, S]
    return np.ascontiguousarray(full.T[None]).astype(np.float32)


# revision 52
# speedup vs baseline: 1.0102x; 1.0102x over previous
"""Trainium2 Bass kernel for nn_PixtralHFVisionModel (8-core TP).

Strategy (Megatron tensor-parallel over 8 NeuronCores, one chip):
  - Patch-embed conv done as matmul (host im2col), replicated on all cores.
  - Activations live TRANSPOSED in SBUF: [hidden(partitions, 8x128), S(free)].
  - Per-core: 2 of 16 heads (q/k/v/o column/row split), 512 of 4096 MLP
    intermediate channels.
  - The sequence is processed as three uniform 512-token chunks
    (C0,C1 = img0, C2 = img1).  o-proj / down-proj partial sums are
    AllReduced per *chunk* (1 MB bf16 each, 6 per layer) so every AR
    hides behind the other chunks' compute; each core keeps the full
    f32 residual stream.
  - Attention per image block (block-diagonal mask is free).  Scores are
    built transposed [kv, q] so softmax-exp output feeds the AV matmul
    directly; denominator via an appended ones-column in the V operand.
  - exp without max subtraction (scores*scale is provably small here).
  - RoPE: rotate-half done as partition-swap DMAs + signed sin table.
  - RMSNorm weights (attn_norm/ffn_norm) are folded into the q/k/v and
    gate/up weights on the host, so the norm apply is a single
    tensor-tensor multiply by the broadcast rstd.
All matmuls bf16 inputs, f32 PSUM accumulation.
"""
import sys

if "/opt/trn_rl_repo" not in sys.path:
    sys.path.insert(0, "/opt/trn_rl_repo")

import numpy as np
import ml_dtypes

BF16 = ml_dtypes.bfloat16
NCORES = 8
HID = 1024
HD = 64
INTER = 4096
NLAYERS = 4
PATCH = 16
MAXSIDE = 64
THETA = 10000.0
EPS = 1e-5
SCALE = HD ** -0.5
GRIDS = [(32, 32), (32, 16)]
S0, S1 = 1024, 512
S = S0 + S1
CH = 512            # tokens per chunk == free-dim matmul size (1 PSUM bank)
KT = HID // 128     # 8 hidden k-tiles
PKT = 768 // 128    # 6 patch k-tiles
MT_I = 512 // 128   # 4 intermediate m-tiles per core
NCHUNK = 3
C_IMG = [0, 0, 1]          # chunk -> image
C_LO = [0, 512, 1024]      # chunk -> global token offset
C_LOC = [0, 512, 0]        # chunk -> offset inside its image
IMG_CHUNKS = [[0, 1], [2]]
NKV = [8, 4]               # kv 128-tiles per image
IMW = [1024, 512]

_CACHE = {}


def _build_nc():
    import concourse.bacc as bacc
    from concourse import tile
    import concourse.mybir as mybir

    dt = mybir.dt
    f32, bf16 = dt.float32, dt.bfloat16
    AF = mybir.ActivationFunctionType
    ALU = mybir.AluOpType

    nc = bacc.Bacc("TRN2", target_bir_lowering=False, debug=False,
                   num_devices=NCORES)
    marks = []

    def mark(label):
        marks.append((label, nc.next_id()))

    def din(name, shape, dtype=bf16):
        return nc.dram_tensor(name, shape, dtype, kind="ExternalInput")

    patchesT_d = din("patchesT", [128, PKT, S])
    convWT_d = din("convWT", [128, PKT, HID])
    eye8_d = din("eye8", [128, 128])
    cos2_d = din("cos2", [128, S])
    sin2_d = din("sin2", [128, S])
    lnw_d = din("lnw", [128, KT], f32)
    wq_d = din("wq", [NLAYERS, 128, KT, 128])
    wk_d = din("wk", [NLAYERS, 128, KT, 128])
    wv2_d = din("wv2", [NLAYERS, 128, KT, 130])
    wo0_d = din("wo0", [NLAYERS, 64, KT, 128])
    wo1_d = din("wo1", [NLAYERS, 64, KT, 128])
    wg_d = din("wg", [NLAYERS, 128, KT, 512])
    wu_d = din("wu", [NLAYERS, 128, KT, 512])
    wd_d = din("wd", [NLAYERS, 128, MT_I, HID])
    out_d = nc.dram_tensor("out", [128, KT, S], f32, kind="ExternalOutput")

    import concourse.bass as bass_mod

    with tile.TileContext(nc) as tc:
        with (
            tc.tile_pool(name="const", bufs=1) as constp,
            tc.tile_pool(name="big", bufs=1) as bigp,
            tc.tile_pool(name="wat", bufs=1) as watp,
            tc.tile_pool(name="wmlp", bufs=1) as wmlpp,
            tc.tile_pool(name="att", bufs=1) as attp,
            tc.tile_pool(name="wrk1", bufs=2) as wrk1p,
            tc.tile_pool(name="wrk2", bufs=2) as wrk2p,
            tc.tile_pool(name="cast", bufs=2) as castp,
            tc.tile_pool(name="stage", bufs=2) as stagep,
            tc.tile_pool(name="dram", bufs=2, space="DRAM") as dramp,
            tc.tile_pool(name="psA", bufs=2, space="PSUM") as psA,
            tc.tile_pool(name="psB", bufs=3, space="PSUM") as psB,
            tc.tile_pool(name="psC", bufs=2, space="PSUM") as psC,
            tc.tile_pool(name="psS", bufs=1, space="PSUM") as psS,
        ):
            # ---- persistent tiles ----
            cos2 = constp.tile([128, S], bf16, tag="cos2")
            sin2 = constp.tile([128, S], bf16, tag="sin2")
            ones1 = constp.tile([128, 1], bf16, tag="ones1")
            onesb = constp.tile([65, 128], f32, tag="onesb")
            onesh = constp.tile([1, 128], bf16, tag="onesh")
            epsc = constp.tile([128, 1], f32, tag="epsc")
            lnw = constp.tile([128, KT], f32, tag="lnw")
            eye8 = constp.tile([128, 128], bf16, tag="eye8")
            nc.sync.dma_start(cos2[:], cos2_d[:])
            nc.sync.dma_start(sin2[:], sin2_d[:])
            nc.sync.dma_start(lnw[:], lnw_d[:])
            nc.sync.dma_start(eye8[:], eye8_d[:])
            nc.gpsimd.memset(ones1[:], 1.0)
            nc.gpsimd.memset(onesb[:], 1.0)
            nc.gpsimd.memset(onesh[:], 1.0)
            nc.gpsimd.memset(epsc[:], EPS)

            def act_raw(out, in_, func, bias=0.0, scale=1.0):
                """activation() without the Rsqrt/Reciprocal accuracy guard."""
                eng = nc.scalar
                inputs = [eng.lower_ap(in_)]
                for arg in (bias, scale, 0.0):
                    if isinstance(arg, bass_mod.AP):
                        inputs.append(eng.lower_ap(arg))
                    else:
                        inputs.append(mybir.ImmediateValue(
                            dtype=f32, value=float(arg)))
                return eng.add_instruction(mybir.InstActivation(
                    name=f"I-{nc.next_id()}", func=func,
                    ins=inputs, outs=[eng.lower_ap(out)]))

            # per-chunk residual: the current AllReduce output tile
            # [128, KT*CH] bf16 (ring-allocated).  resid/8 is re-injected
            # into every o/down projection via an eye/8 matmul so the AR
            # output IS the new residual (no post-AR adds).
            rarr = [None] * NCHUNK

            def new_rarr(c):
                t = stagep.tile([128, KT * CH], bf16, tag="arr", bufs=5,
                                name=f"arr{c}")
                rarr[c] = t
                return t

            def rsl(kt):
                return slice(kt * CH, (kt + 1) * CH)

            xnorms = [[bigp.tile([128, CH], bf16, tag=f"xn{c}_{k}",
                                 name=f"xn{c}_{k}") for k in range(KT)]
                      for c in range(NCHUNK)]
            hmlps = [[bigp.tile([128, CH], bf16, tag=f"hm{c}_{m}",
                                name=f"hm{c}_{m}") for m in range(MT_I)]
                     for c in range(NCHUNK)]
            # per-image K (rope'd) and V2 (kv-major with ones columns)
            kts = [attp.tile([128, IMW[i], ], bf16, tag=f"kt{i}",
                             name=f"kts{i}") for i in range(2)]
            v2s = [attp.tile([128, NKV[i], 130], bf16, tag=f"v2{i}",
                             name=f"v2s{i}") for i in range(2)]
            qts = [attp.tile([128, CH], bf16, tag=f"qt{c}",
                             name=f"qts{c}") for c in range(NCHUNK)]
            for i in range(2):
                nc.gpsimd.memset(v2s[i][:, :, 64:65], 1.0)
                nc.gpsimd.memset(v2s[i][:, :, 129:130], 1.0)

            def rms_rstd(c):
                """rstdb [128,CH] f32 PSUM = bcast(rsqrt(mean(resid^2)+eps)).

                Squares alternate vector/gpsimd so they pipeline; the
                partition-broadcast of rstd is a ones outer-product matmul
                (avoids a gpsimd hop on the post-AR latency chain)."""
                src = rarr[c]
                pss = psS.tile([128, CH], f32, tag="pss")
                for kt in range(KT):
                    sq = castp.tile([128, CH], bf16, tag="sq")
                    eng = nc.vector if kt % 2 == 0 else nc.gpsimd
                    eng.tensor_mul(sq[:], src[:, rsl(kt)], src[:, rsl(kt)])
                    nc.tensor.matmul(pss[0:1, :], lhsT=ones1[:], rhs=sq[:],
                                     start=(kt == 0), stop=(kt == KT - 1))
                rstd0 = wrk1p.tile([1, CH], bf16, tag="rstd0", bufs=1)
                act_raw(rstd0[:], pss[0:1, :], AF.Rsqrt,
                        bias=epsc[0:1, :], scale=1.0 / HID)
                nc.tensor.matmul(pss[:], lhsT=onesh[:], rhs=rstd0[:],
                                 start=True, stop=True)
                return pss

            def norm(c):
                """xnorm_c = bf16(resid_c * rstd)  (norm wt folded into W)."""
                rstdb = rms_rstd(c)
                src = rarr[c]
                for kt in range(KT):
                    nc.vector.tensor_mul(xnorms[c][kt][:], src[:, rsl(kt)],
                                         rstdb[:])

            def ln_pre(c):
                """resid_c = bf16(resid_c * lnw * rstd)  (once, at start)."""
                rstdb = rms_rstd(c)
                src = rarr[c]
                dst = new_rarr(c)
                for kt in range(KT):
                    nc.vector.scalar_tensor_tensor(
                        dst[:, rsl(kt)], src[:, rsl(kt)], lnw[:, kt:kt + 1],
                        rstdb[:], ALU.mult, ALU.mult)

            def readback(c, arout):
                """new resid_c = AllReduce output (two split DMAs)."""
                arr = new_rarr(c)
                H = KT * CH // 2
                nc.sync.dma_start(arr[:, 0:H], arout[:, 0:H])
                nc.sync.dma_start(arr[:, H:], arout[:, H:])

            def rope(dst, lo):
                """in-place rope on dst [128, CH] at global offset lo."""
                rot = wrk1p.tile([128, CH], bf16, tag="rot")
                t1 = wrk1p.tile([128, CH], bf16, tag="t1")
                for h in range(2):
                    b = h * 64
                    nc.sync.dma_start(rot[b:b + 32, :], dst[b + 32:b + 64, :])
                    nc.sync.dma_start(rot[b + 32:b + 64, :], dst[b:b + 32, :])
                nc.vector.tensor_mul(t1[:], dst[:], cos2[:, lo:lo + CH])
                nc.vector.tensor_mul(rot[:], rot[:], sin2[:, lo:lo + CH])
                nc.vector.tensor_add(dst[:], t1[:], rot[:])

            def k_proj(c, wk):
                img, loc, lo = C_IMG[c], C_LOC[c], C_LO[c]
                psq = psA.tile([128, CH], f32, tag="psx")
                for kt in range(KT):
                    nc.tensor.matmul(psq[:], lhsT=wk[:, kt, :],
                                     rhs=xnorms[c][kt][:],
                                     start=(kt == 0), stop=(kt == KT - 1))
                dst = kts[img][:, loc:loc + CH]
                nc.scalar.activation(dst, psq[:], AF.Copy)
                rope(dst, lo)

            def q_proj(c, wq):
                psq = psA.tile([128, CH], f32, tag="psx")
                for kt in range(KT):
                    nc.tensor.matmul(psq[:], lhsT=wq[:, kt, :],
                                     rhs=xnorms[c][kt][:],
                                     start=(kt == 0), stop=(kt == KT - 1))
                nc.scalar.activation(qts[c][:], psq[:], AF.Copy)
                rope(qts[c], C_LO[c])

            def v_proj(img, wv2):
                # ones columns 64/129 were preset once at startup; copy only
                # the V halves so no per-layer memset gates the AV matmuls
                for kv in range(NKV[img]):
                    c = IMG_CHUNKS[img][kv // 4]
                    loc = (kv % 4) * 128
                    psv = psB.tile([128, 512], f32, tag="psb")
                    for kt in range(KT):
                        nc.tensor.matmul(
                            psv[:, 0:130],
                            lhsT=xnorms[c][kt][:, loc:loc + 128],
                            rhs=wv2[:, kt, :],
                            start=(kt == 0), stop=(kt == KT - 1))
                    nc.scalar.activation(v2s[img][:, kv, 0:64], psv[:, 0:64],
                                         AF.Copy)
                    nc.scalar.activation(v2s[img][:, kv, 65:129],
                                         psv[:, 65:129], AF.Copy)

            def attn(c, wo0, wo1):
                """attention + o-proj for one chunk -> AR input dram tile."""
                img = C_IMG[c]
                kt_t, v2, qt = kts[img], v2s[img], qts[c]
                hsls = [slice(h * 64, (h + 1) * 64) for h in range(2)]
                vsls = [slice(h * 65, h * 65 + 65) for h in range(2)]
                # both heads interleaved as paired streams so each AV's exp
                # finishes during the other head's scores matmul
                psavs = [psC.tile([65, CH], f32, tag="psav",
                                  name=f"psav{h}") for h in range(2)]
                # one-kv-tile lookahead: scores for tile i+1 are issued
                # before the AV matmuls of tile i, so the exp latency never
                # bubbles the PE queue
                nkv = NKV[img]
                pend = None
                for i in range(nkv):
                    pts = []
                    for h in range(2):
                        pss = psB.tile([128, 512], f32, tag="psb")
                        nc.tensor.matmul(
                            pss[:, 0:CH],
                            lhsT=kt_t[hsls[h], i * 128:(i + 1) * 128],
                            rhs=qt[hsls[h], :],
                            start=True, stop=True)
                        pt = castp.tile([128, CH], bf16, tag="pt",
                                        bufs=4, name=f"pt{h}")
                        nc.scalar.activation(pt[:], pss[:, 0:CH],
                                             AF.Exp, scale=SCALE)
                        pts.append(pt)
                    if pend is not None:
                        for h in range(2):
                            nc.tensor.matmul(
                                psavs[h][:], lhsT=v2[:, i - 1, vsls[h]],
                                rhs=pend[h][:],
                                start=(i == 1), stop=False)
                    pend = pts
                for h in range(2):
                    nc.tensor.matmul(
                        psavs[h][:], lhsT=v2[:, nkv - 1, vsls[h]],
                        rhs=pend[h][:],
                        start=(nkv == 1), stop=True)
                ots = []
                for h in range(2):
                    rec = wrk2p.tile([65, CH], f32, tag="rec")
                    nc.vector.reciprocal(rec[64:65, :], psavs[h][64:65, :])
                    avs = wrk2p.tile([64, CH], bf16, tag=f"avs{h}",
                                     name=f"avs{h}")
                    nc.scalar.activation(avs[:], psavs[h][0:64, :], AF.Copy)
                    ob = psB.tile([128, 512], f32, tag="psb")
                    nc.tensor.matmul(ob[0:64, 0:CH],
                                     lhsT=onesb[64:65, 0:64],
                                     rhs=rec[64:65, :],
                                     start=True, stop=True)
                    ot = wrk2p.tile([64, CH], bf16, tag=f"ot{h}",
                                    name=f"ot{h}")
                    nc.vector.tensor_mul(ot[:], avs[:], ob[0:64, 0:CH])
                    ots.append(ot)
                # o-projection + resid/8 -> bf16 -> per-chunk AllReduce
                dsm = stagep.tile([128, KT * CH], bf16, tag="dsm",
                                  name=f"dsa{c}")
                res = rarr[c]
                for kt in range(KT):
                    pso = psA.tile([128, CH], f32, tag="psx")
                    nc.tensor.matmul(pso[:], lhsT=wo0[:, kt, :], rhs=ots[0][:],
                                     start=True, stop=False)
                    nc.tensor.matmul(pso[:], lhsT=wo1[:, kt, :], rhs=ots[1][:],
                                     start=False, stop=False)
                    nc.tensor.matmul(pso[:], lhsT=eye8[:],
                                     rhs=res[:, rsl(kt)],
                                     start=False, stop=True)
                    nc.vector.tensor_copy(dsm[:, rsl(kt)], pso[:])
                return start_ar(c, "a", dsm)

            def start_ar(c, tagc, dsm):
                arin = dramp.tile([128, KT * CH], bf16, tag=f"ari{tagc}{c}",
                                  name=f"ari{tagc}{c}")
                arout = dramp.tile([128, KT * CH], bf16, tag=f"aro{tagc}{c}",
                                   name=f"aro{tagc}{c}", addr_space="Shared")
                nc.sync.dma_start(arin[:], dsm[:])
                nc.gpsimd.collective_compute(
                    "AllReduce", ALU.add,
                    ins=[arin.opt()], outs=[arout.opt()],
                    replica_groups=[list(range(NCORES))])
                return arout

            def mlp(c, wg, wu, wd):
                for mt in range(MT_I):
                    msl = slice(mt * 128, (mt + 1) * 128)
                    psg = psA.tile([128, CH], f32, tag="psx")
                    for kt in range(KT):
                        nc.tensor.matmul(
                            psg[:], lhsT=wg[:, kt, msl], rhs=xnorms[c][kt][:],
                            start=(kt == 0), stop=(kt == KT - 1))
                    gts = castp.tile([128, CH], bf16, tag="gts", bufs=2)
                    nc.scalar.activation(gts[:], psg[:], AF.Silu)
                    psu = psB.tile([128, 512], f32, tag="psb")
                    for kt in range(KT):
                        nc.tensor.matmul(
                            psu[:, 0:CH], lhsT=wu[:, kt, msl],
                            rhs=xnorms[c][kt][:],
                            start=(kt == 0), stop=(kt == KT - 1))
                    nc.vector.tensor_mul(hmlps[c][mt][:], gts[:],
                                         psu[:, 0:CH])
                dsm = stagep.tile([128, KT * CH], bf16, tag="dsm",
                                  name=f"dsm{c}")
                res = rarr[c]
                for kt in range(KT):
                    psd = psA.tile([128, CH], f32, tag="psx")
                    for mt in range(MT_I):
                        nc.tensor.matmul(
                            psd[:], lhsT=wd[:, mt, kt * 128:(kt + 1) * 128],
                            rhs=hmlps[c][mt][:],
                            start=(mt == 0), stop=False)
                    nc.tensor.matmul(psd[:], lhsT=eye8[:],
                                     rhs=res[:, rsl(kt)],
                                     start=False, stop=True)
                    if kt % 2 == 0:
                        nc.scalar.activation(dsm[:, rsl(kt)], psd[:], AF.Copy)
                    else:
                        nc.vector.tensor_copy(dsm[:, rsl(kt)], psd[:])
                return start_ar(c, "m", dsm)

            # ---- conv patch embed (replicated, streamed) + ln_pre ----
            with tc.tile_pool(name="convp", bufs=2) as convp:
                for c in range(NCHUNK):
                    gco = C_LO[c]
                    pch = convp.tile([128, PKT, CH], bf16, tag="pch")
                    nc.sync.dma_start(pch[:], patchesT_d[:, :, gco:gco + CH])
                    cr = new_rarr(c)
                    for kt in range(KT):
                        cwt = convp.tile([128, PKT, 128], bf16, tag="cwt")
                        nc.sync.dma_start(
                            cwt[:], convWT_d[:, :, kt * 128:(kt + 1) * 128])
                        psx = psA.tile([128, CH], f32, tag="psx")
                        for pk in range(PKT):
                            nc.tensor.matmul(
                                psx[:], lhsT=cwt[:, pk, :],
                                rhs=pch[:, pk, :],
                                start=(pk == 0), stop=(pk == PKT - 1))
                        nc.scalar.activation(cr[:, rsl(kt)], psx[:], AF.Copy)
                    ln_pre(c)

            # ---- transformer layers (3-chunk pipelined stream) ----
            ar_m = [None, None, None]
            for l in range(NLAYERS):
                wq = watp.tile([128, KT, 128], bf16, tag="wq")
                wk = watp.tile([128, KT, 128], bf16, tag="wk")
                wv2 = watp.tile([128, KT, 130], bf16, tag="wv2")
                wo0 = watp.tile([64, KT, 128], bf16, tag="wo0")
                wo1 = watp.tile([64, KT, 128], bf16, tag="wo1")
                nc.sync.dma_start(wq[:], wq_d[l])
                nc.sync.dma_start(wk[:], wk_d[l])
                nc.sync.dma_start(wv2[:], wv2_d[l])
                nc.sync.dma_start(wo0[:], wo0_d[l])
                nc.sync.dma_start(wo1[:], wo1_d[l])
                wg = wmlpp.tile([128, KT, 512], bf16, tag="wg")
                wu = wmlpp.tile([128, KT, 512], bf16, tag="wu")
                wd = wmlpp.tile([128, MT_I, HID], bf16, tag="wd")
                nc.sync.dma_start(wg[:], wg_d[l])
                nc.sync.dma_start(wu[:], wu_d[l])
                nc.sync.dma_start(wd[:], wd_d[l])

                # attn norms + q/k for chunks 0,1 first so attention on img0
                # can start while chunk 2's AR from the previous layer lands
                # readbacks are emitted one chunk ahead of their consumer so
                # they never sit behind a later arin-write in the sync queue
                if ar_m[2] is not None:
                    readback(2, ar_m[2])
                for c in (0, 1):
                    mark(f"L{l}nA{c}")
                    norm(c)
                    mark(f"L{l}kq{c}")
                    k_proj(c, wk)
                    q_proj(c, wq)
                mark(f"L{l}v0")
                v_proj(0, wv2)
                mark(f"L{l}nA2")
                norm(2)
                mark(f"L{l}kq2")
                k_proj(2, wk)
                q_proj(2, wq)
                mark(f"L{l}v1")
                v_proj(1, wv2)
                ar_a = []
                for c in range(NCHUNK):
                    mark(f"L{l}at{c}")
                    ar_a.append(attn(c, wo0, wo1))
                    if c >= 1:
                        readback(c - 1, ar_a[c - 1])
                for c in range(NCHUNK):
                    mark(f"L{l}nF{c}")
                    norm(c)
                    mark(f"L{l}ml{c}")
                    if c >= 1:
                        readback(c - 1, ar_m[c - 1])
                    ar_m[c] = mlp(c, wg, wu, wd)
                    if c == 0:
                        readback(2, ar_a[2])

            def wout(c):
                lo = C_LO[c]
                for kt in range(KT):
                    fin = wrk1p.tile([128, CH], f32, tag="fin", bufs=2)
                    nc.scalar.activation(fin[:], rarr[c][:, rsl(kt)], AF.Copy)
                    nc.sync.dma_start(out_d[:, kt, lo:lo + CH], fin[:])

            wout(0)
            wout(1)
            readback(2, ar_m[2])
            wout(2)

    mark("end")
    nc.compile()
    import json as _json
    with open("/tmp/phase_marks.json", "w") as f:
        _json.dump(marks, f)
    return nc


# ---------------- host-side prep ----------------

def _im2col(img):
    C, H, W = img.shape
    h, w = H // PATCH, W // PATCH
    p = img.reshape(C, h, PATCH, w, PATCH).transpose(1, 3, 0, 2, 4)
    return p.reshape(h * w, C * PATCH * PATCH)


def _rope_tables():
    freqs = 1.0 / THETA ** (np.arange(0, HD, 2, dtype=np.float64) / HD)
    fh = np.outer(np.arange(MAXSIDE, dtype=np.float64), freqs[::2])
    fw = np.outer(np.arange(MAXSIDE, dtype=np.float64), freqs[1::2])
    pids = np.concatenate([
        (np.arange(h)[:, None] * MAXSIDE + np.arange(w)[None, :]).reshape(-1)
        for h, w in GRIDS])
    inv = np.concatenate([
        np.broadcast_to(fh[:, None, :], (MAXSIDE, MAXSIDE, HD // 4)),
        np.broadcast_to(fw[None, :, :], (MAXSIDE, MAXSIDE, HD // 4))],
        axis=-1).reshape(-1, HD // 2)
    inv = np.concatenate([inv, inv], axis=-1)
    emb = inv[pids]                                   # [S, 64]
    cosT = np.cos(emb).T.astype(np.float32)           # [64, S]
    sinT = np.sin(emb).T.astype(np.float32)
    sinTs = np.concatenate([-sinT[:32], sinT[32:]], axis=0)
    cos2 = np.concatenate([cosT, cosT], axis=0).astype(BF16)
    sin2 = np.concatenate([sinTs, sinTs], axis=0).astype(BF16)
    return np.ascontiguousarray(cos2), np.ascontiguousarray(sin2)


def _ktile(a, last):
    """[L, 1024, last] -> [L, 128, kt, last] (partition-major k-tiles)."""
    L = a.shape[0]
    return np.ascontiguousarray(
        a.reshape(L, -1, 128, last).transpose(0, 2, 1, 3))


def _prep(inputs):
    f32 = np.float32
    patches = np.concatenate([
        _im2col(np.asarray(inputs["img0"], f32)),
        _im2col(np.asarray(inputs["img1"], f32))])          # [S, 768]
    patchesT = np.ascontiguousarray(
        patches.T.reshape(PKT, 128, S).transpose(1, 0, 2)).astype(BF16)
    cw = np.asarray(inputs["conv_w"], f32).reshape(HID, 768)
    convWT = np.ascontiguousarray(
        cw.T.reshape(PKT, 128, HID).transpose(1, 0, 2)).astype(BF16)
    cos2, sin2 = _rope_tables()
    lnw = np.ascontiguousarray(
        np.asarray(inputs["ln_pre_w"], f32).reshape(KT, 128).T)

    anw = np.asarray(inputs["attn_norm_w"], f32)              # [4, 1024]
    fnw = np.asarray(inputs["ffn_norm_w"], f32)
    # fold the norm weights into the following projections (input dim)
    qwT = np.asarray(inputs["q_w"], f32).transpose(0, 2, 1) * anw[:, :, None]
    kwT = np.asarray(inputs["k_w"], f32).transpose(0, 2, 1) * anw[:, :, None]
    vwT = np.asarray(inputs["v_w"], f32).transpose(0, 2, 1) * anw[:, :, None]
    owT = np.asarray(inputs["o_w"], f32).transpose(0, 2, 1)   # [4, d, e]
    gwT = np.asarray(inputs["gate_w"], f32).transpose(0, 2, 1) * fnw[:, :, None]
    uwT = np.asarray(inputs["up_w"], f32).transpose(0, 2, 1) * fnw[:, :, None]
    dwT = np.asarray(inputs["down_w"], f32).transpose(0, 2, 1)  # [4, I, out]

    eye8 = (np.eye(128, dtype=f32) * 0.125).astype(BF16)
    common = dict(patchesT=patchesT, convWT=convWT, cos2=cos2, sin2=sin2,
                  lnw=lnw, eye8=eye8)
    in_maps = []
    for c in range(NCORES):
        esl = slice(c * 128, (c + 1) * 128)
        isl = slice(c * 512, (c + 1) * 512)
        wv = vwT[:, :, esl].astype(BF16)                      # [4, 1024, 128]
        wv2 = np.zeros((NLAYERS, HID, 130), BF16)
        wv2[:, :, 0:64] = wv[:, :, 0:64]
        wv2[:, :, 65:129] = wv[:, :, 64:128]
        wo = owT[:, esl, :]                                   # [4, 128, 1024]
        m = dict(
            wq=_ktile(qwT[:, :, esl].astype(BF16), 128),
            wk=_ktile(kwT[:, :, esl].astype(BF16), 128),
            wv2=_ktile(wv2, 130),
            wo0=np.ascontiguousarray(
                wo[:, 0:64, :].reshape(NLAYERS, 64, KT, 128)).astype(BF16),
            wo1=np.ascontiguousarray(
                wo[:, 64:128, :].reshape(NLAYERS, 64, KT, 128)).astype(BF16),
            wg=_ktile(gwT[:, :, isl].astype(BF16), 512),
            wu=_ktile(uwT[:, :, isl].astype(BF16), 512),
            wd=np.ascontiguousarray(
                dwT[:, isl, :].reshape(NLAYERS, MT_I, 128, HID)
                .transpose(0, 2, 1, 3)).astype(BF16),
            **common)
        in_maps.append(m)
    return in_maps


LAST_RESULTS = None
TRACE = False


def _install_ntff_hook():
    """The RL container's antenv lacks axon_hooks; recreate it so
    trace=True can capture NTFF profiles through the axon terminal."""
    import types
    import antenv

    if hasattr(antenv, "axon_hooks"):
        return
    mod = types.ModuleType("antenv.axon_hooks")
    holder = [None]
    mod.set_axon_ntff_profile_hook = lambda h: holder.__setitem__(0, h)
    mod.get_axon_ntff_profile_hook = lambda: holder[0]
    sys.modules["antenv.axon_hooks"] = mod
    antenv.axon_hooks = mod
    if "/root/.axon_site" not in sys.path:
        sys.path.insert(0, "/root/.axon_site")
    try:
        from trn_agent_boot.trn_boot import _ntff_profile_via_ctypes
        mod.set_axon_ntff_profile_hook(
            _ntff_profile_via_ctypes("/opt/axon/libaxon_pjrt.so"))
    except Exception as e:  # pragma: no cover
        print("ntff hook install failed:", e)


def kernel(**inputs):
    global LAST_RESULTS
    from concourse import bass_utils

    if TRACE:
        _install_ntff_hook()

    if "nc" not in _CACHE:
        _CACHE["nc"] = _build_nc()
    nc = _CACHE["nc"]
    in_maps = _prep(inputs)
    res = bass_utils.run_bass_kernel_spmd(
        nc, in_maps, core_ids=list(range(NCORES)), trace=TRACE)
    LAST_RESULTS = res
    out = res.results[0]["out"]                 # [128, KT, S] f32
    full = out.transpose(1, 0, 2).reshape(HID, S)   # [hid, S]
    return np.ascontiguousarray(full.T[None]).astype(np.float32)


# revision 54
# speedup vs baseline: 1.0434x; 1.0328x over previous
"""Trainium2 Bass kernel for nn_PixtralHFVisionModel (8-core TP).

Strategy (Megatron tensor-parallel over 8 NeuronCores, one chip):
  - Patch-embed conv done as matmul (host im2col), replicated on all cores.
  - Activations live TRANSPOSED in SBUF: [hidden(partitions, 8x128), S(free)].
  - Per-core: 2 of 16 heads (q/k/v/o column/row split), 512 of 4096 MLP
    intermediate channels.
  - The sequence is processed as three uniform 512-token chunks
    (C0,C1 = img0, C2 = img1).  o-proj / down-proj partial sums are
    AllReduced per *chunk* (1 MB bf16 each, 6 per layer) so every AR
    hides behind the other chunks' compute; each core keeps the full
    f32 residual stream.
  - Attention per image block (block-diagonal mask is free).  Scores are
    built transposed [kv, q] so softmax-exp output feeds the AV matmul
    directly; denominator via an appended ones-column in the V operand.
  - exp without max subtraction (scores*scale is provably small here).
  - RoPE: rotate-half done as partition-swap DMAs + signed sin table.
  - RMSNorm weights (attn_norm/ffn_norm) are folded into the q/k/v and
    gate/up weights on the host, so the norm apply is a single
    tensor-tensor multiply by the broadcast rstd.
All matmuls bf16 inputs, f32 PSUM accumulation.
"""
import sys

if "/opt/trn_rl_repo" not in sys.path:
    sys.path.insert(0, "/opt/trn_rl_repo")

import numpy as np
import ml_dtypes

BF16 = ml_dtypes.bfloat16
NCORES = 8
HID = 1024
HD = 64
INTER = 4096
NLAYERS = 4
PATCH = 16
MAXSIDE = 64
THETA = 10000.0
EPS = 1e-5
SCALE = HD ** -0.5
GRIDS = [(32, 32), (32, 16)]
S0, S1 = 1024, 512
S = S0 + S1
CH = 512            # tokens per chunk == free-dim matmul size (1 PSUM bank)
KT = HID // 128     # 8 hidden k-tiles
PKT = 768 // 128    # 6 patch k-tiles
MT_I = 512 // 128   # 4 intermediate m-tiles per core
NCHUNK = 3
C_IMG = [0, 0, 1]          # chunk -> image
C_LO = [0, 512, 1024]      # chunk -> global token offset
C_LOC = [0, 512, 0]        # chunk -> offset inside its image
IMG_CHUNKS = [[0, 1], [2]]
NKV = [8, 4]               # kv 128-tiles per image
IMW = [1024, 512]

_CACHE = {}


def _build_nc():
    import concourse.bacc as bacc
    from concourse import tile
    import concourse.mybir as mybir

    dt = mybir.dt
    f32, bf16 = dt.float32, dt.bfloat16
    AF = mybir.ActivationFunctionType
    ALU = mybir.AluOpType

    nc = bacc.Bacc("TRN2", target_bir_lowering=False, debug=False,
                   num_devices=NCORES)
    marks = []

    def mark(label):
        marks.append((label, nc.next_id()))

    def din(name, shape, dtype=bf16):
        return nc.dram_tensor(name, shape, dtype, kind="ExternalInput")

    patchesT_d = din("patchesT", [128, PKT, S])
    convWT_d = din("convWT", [128, PKT, HID])
    eye8_d = din("eye8", [128, 128])
    cos2_d = din("cos2", [128, S])
    sin2_d = din("sin2", [128, S])
    lnw_d = din("lnw", [128, KT], f32)
    wq_d = din("wq", [NLAYERS, 128, KT, 128])
    wk_d = din("wk", [NLAYERS, 128, KT, 128])
    wv2_d = din("wv2", [NLAYERS, 128, KT, 130])
    wo0_d = din("wo0", [NLAYERS, 64, KT, 128])
    wo1_d = din("wo1", [NLAYERS, 64, KT, 128])
    wg_d = din("wg", [NLAYERS, 128, KT, 512])
    wu_d = din("wu", [NLAYERS, 128, KT, 512])
    wd_d = din("wd", [NLAYERS, 128, MT_I, HID])
    out_d = nc.dram_tensor("out", [128, KT, S], f32, kind="ExternalOutput")

    import concourse.bass as bass_mod

    with tile.TileContext(nc) as tc:
        with (
            tc.tile_pool(name="const", bufs=1) as constp,
            tc.tile_pool(name="big", bufs=1) as bigp,
            tc.tile_pool(name="wat", bufs=1) as watp,
            tc.tile_pool(name="wmlp", bufs=1) as wmlpp,
            tc.tile_pool(name="att", bufs=1) as attp,
            tc.tile_pool(name="wrk1", bufs=2) as wrk1p,
            tc.tile_pool(name="wrk2", bufs=2) as wrk2p,
            tc.tile_pool(name="cast", bufs=2) as castp,
            tc.tile_pool(name="stage", bufs=2) as stagep,
            tc.tile_pool(name="dram", bufs=2, space="DRAM") as dramp,
            tc.tile_pool(name="psA", bufs=2, space="PSUM") as psA,
            tc.tile_pool(name="psB", bufs=3, space="PSUM") as psB,
            tc.tile_pool(name="psC", bufs=2, space="PSUM") as psC,
            tc.tile_pool(name="psS", bufs=1, space="PSUM") as psS,
        ):
            # ---- persistent tiles ----
            cos2 = constp.tile([128, S], bf16, tag="cos2")
            sin2 = constp.tile([128, S], bf16, tag="sin2")
            ones1 = constp.tile([128, 1], bf16, tag="ones1")
            onesb = constp.tile([65, 128], f32, tag="onesb")
            onesh = constp.tile([1, 128], bf16, tag="onesh")
            epsc = constp.tile([128, 1], f32, tag="epsc")
            lnw = constp.tile([128, KT], f32, tag="lnw")
            eye8 = constp.tile([128, 128], bf16, tag="eye8")
            nc.sync.dma_start(cos2[:], cos2_d[:])
            nc.sync.dma_start(sin2[:], sin2_d[:])
            nc.sync.dma_start(lnw[:], lnw_d[:])
            nc.sync.dma_start(eye8[:], eye8_d[:])
            nc.gpsimd.memset(ones1[:], 1.0)
            nc.gpsimd.memset(onesb[:], 1.0)
            nc.gpsimd.memset(onesh[:], 1.0)
            nc.gpsimd.memset(epsc[:], EPS)

            def act_raw(out, in_, func, bias=0.0, scale=1.0):
                """activation() without the Rsqrt/Reciprocal accuracy guard."""
                eng = nc.scalar
                inputs = [eng.lower_ap(in_)]
                for arg in (bias, scale, 0.0):
                    if isinstance(arg, bass_mod.AP):
                        inputs.append(eng.lower_ap(arg))
                    else:
                        inputs.append(mybir.ImmediateValue(
                            dtype=f32, value=float(arg)))
                return eng.add_instruction(mybir.InstActivation(
                    name=f"I-{nc.next_id()}", func=func,
                    ins=inputs, outs=[eng.lower_ap(out)]))

            # per-chunk residual: the current AllReduce output tile
            # [128, KT*CH] bf16 (ring-allocated).  resid/8 is re-injected
            # into every o/down projection via an eye/8 matmul so the AR
            # output IS the new residual (no post-AR adds).
            rarr = [None] * NCHUNK

            def new_rarr(c):
                t = stagep.tile([128, KT * CH], bf16, tag="arr", bufs=5,
                                name=f"arr{c}")
                rarr[c] = t
                return t

            def rsl(kt):
                return slice(kt * CH, (kt + 1) * CH)

            xnorms = [[bigp.tile([128, CH], bf16, tag=f"xn{c}_{k}",
                                 name=f"xn{c}_{k}") for k in range(KT)]
                      for c in range(NCHUNK)]
            hmlps = [[bigp.tile([128, CH], bf16, tag=f"hm{c}_{m}",
                                name=f"hm{c}_{m}") for m in range(MT_I)]
                     for c in range(NCHUNK)]
            # per-image K (rope'd) and V2 (kv-major with ones columns)
            kts = [attp.tile([128, IMW[i], ], bf16, tag=f"kt{i}",
                             name=f"kts{i}") for i in range(2)]
            v2s = [attp.tile([128, NKV[i], 130], bf16, tag=f"v2{i}",
                             name=f"v2s{i}") for i in range(2)]
            qts = [attp.tile([128, CH], bf16, tag=f"qt{c}",
                             name=f"qts{c}") for c in range(NCHUNK)]
            for i in range(2):
                nc.gpsimd.memset(v2s[i][:, :, 64:65], 1.0)
                nc.gpsimd.memset(v2s[i][:, :, 129:130], 1.0)

            def rms_rstd(c):
                """rstdb [128,CH] f32 PSUM = bcast(rsqrt(mean(resid^2)+eps)).

                Squares alternate vector/gpsimd so they pipeline; the
                partition-broadcast of rstd is a ones outer-product matmul
                (avoids a gpsimd hop on the post-AR latency chain)."""
                src = rarr[c]
                pss = psS.tile([128, CH], f32, tag="pss")
                for kt in range(KT):
                    sq = castp.tile([128, CH], bf16, tag="sq")
                    eng = nc.vector if kt % 2 == 0 else nc.gpsimd
                    eng.tensor_mul(sq[:], src[:, rsl(kt)], src[:, rsl(kt)])
                    nc.tensor.matmul(pss[0:1, :], lhsT=ones1[:], rhs=sq[:],
                                     start=(kt == 0), stop=(kt == KT - 1))
                rstd0 = wrk1p.tile([1, CH], bf16, tag="rstd0", bufs=1)
                act_raw(rstd0[:], pss[0:1, :], AF.Rsqrt,
                        bias=epsc[0:1, :], scale=1.0 / HID)
                nc.tensor.matmul(pss[:], lhsT=onesh[:], rhs=rstd0[:],
                                 start=True, stop=True)
                return pss

            def norm(c):
                """xnorm_c = bf16(resid_c * rstd)  (norm wt folded into W)."""
                rstdb = rms_rstd(c)
                src = rarr[c]
                for kt in range(KT):
                    nc.vector.tensor_mul(xnorms[c][kt][:], src[:, rsl(kt)],
                                         rstdb[:])

            def ln_pre(c):
                """resid_c = bf16(resid_c * lnw * rstd)  (once, at start)."""
                rstdb = rms_rstd(c)
                src = rarr[c]
                dst = new_rarr(c)
                for kt in range(KT):
                    nc.vector.scalar_tensor_tensor(
                        dst[:, rsl(kt)], src[:, rsl(kt)], lnw[:, kt:kt + 1],
                        rstdb[:], ALU.mult, ALU.mult)

            def readback(c, arout):
                """new resid_c = AllReduce output (two split DMAs)."""
                arr = new_rarr(c)
                H = KT * CH // 2
                nc.sync.dma_start(arr[:, 0:H], arout[:, 0:H])
                nc.sync.dma_start(arr[:, H:], arout[:, H:])

            def rope(dst, lo):
                """in-place rope on dst [128, CH] at global offset lo."""
                rot = wrk1p.tile([128, CH], bf16, tag="rot")
                t1 = wrk1p.tile([128, CH], bf16, tag="t1")
                for h in range(2):
                    b = h * 64
                    nc.sync.dma_start(rot[b:b + 32, :], dst[b + 32:b + 64, :])
                    nc.sync.dma_start(rot[b + 32:b + 64, :], dst[b:b + 32, :])
                nc.vector.tensor_mul(t1[:], dst[:], cos2[:, lo:lo + CH])
                nc.vector.tensor_mul(rot[:], rot[:], sin2[:, lo:lo + CH])
                nc.vector.tensor_add(dst[:], t1[:], rot[:])

            def k_proj(c, wk):
                img, loc, lo = C_IMG[c], C_LOC[c], C_LO[c]
                psq = psA.tile([128, CH], f32, tag="psx")
                for kt in range(KT):
                    nc.tensor.matmul(psq[:], lhsT=wk[:, kt, :],
                                     rhs=xnorms[c][kt][:],
                                     start=(kt == 0), stop=(kt == KT - 1))
                dst = kts[img][:, loc:loc + CH]
                nc.scalar.activation(dst, psq[:], AF.Copy)
                rope(dst, lo)

            def q_proj(c, wq):
                psq = psA.tile([128, CH], f32, tag="psx")
                for kt in range(KT):
                    nc.tensor.matmul(psq[:], lhsT=wq[:, kt, :],
                                     rhs=xnorms[c][kt][:],
                                     start=(kt == 0), stop=(kt == KT - 1))
                nc.scalar.activation(qts[c][:], psq[:], AF.Copy)
                rope(qts[c], C_LO[c])

            def v_proj(img, wv2):
                # ones columns 64/129 were preset once at startup; copy only
                # the V halves so no per-layer memset gates the AV matmuls
                for kv in range(NKV[img]):
                    c = IMG_CHUNKS[img][kv // 4]
                    loc = (kv % 4) * 128
                    psv = psB.tile([128, 512], f32, tag="psb")
                    for kt in range(KT):
                        nc.tensor.matmul(
                            psv[:, 0:130],
                            lhsT=xnorms[c][kt][:, loc:loc + 128],
                            rhs=wv2[:, kt, :],
                            start=(kt == 0), stop=(kt == KT - 1))
                    nc.scalar.activation(v2s[img][:, kv, 0:64], psv[:, 0:64],
                                         AF.Copy)
                    nc.scalar.activation(v2s[img][:, kv, 65:129],
                                         psv[:, 65:129], AF.Copy)

            def attn(c, wo0, wo1):
                """attention + o-proj for one chunk -> AR input dram tile."""
                img = C_IMG[c]
                kt_t, v2, qt = kts[img], v2s[img], qts[c]
                hsls = [slice(h * 64, (h + 1) * 64) for h in range(2)]
                vsls = [slice(h * 65, h * 65 + 65) for h in range(2)]
                # both heads interleaved as paired streams so each AV's exp
                # finishes during the other head's scores matmul
                psavs = [psC.tile([65, CH], f32, tag="psav",
                                  name=f"psav{h}") for h in range(2)]
                # one-kv-tile lookahead: scores for tile i+1 are issued
                # before the AV matmuls of tile i, so the exp latency never
                # bubbles the PE queue
                nkv = NKV[img]
                pend = None
                for i in range(nkv):
                    pts = []
                    for h in range(2):
                        pss = psB.tile([128, 512], f32, tag="psb")
                        nc.tensor.matmul(
                            pss[:, 0:CH],
                            lhsT=kt_t[hsls[h], i * 128:(i + 1) * 128],
                            rhs=qt[hsls[h], :],
                            start=True, stop=True)
                        pt = castp.tile([128, CH], bf16, tag="pt",
                                        bufs=4, name=f"pt{h}")
                        nc.scalar.activation(pt[:], pss[:, 0:CH],
                                             AF.Exp, scale=SCALE)
                        pts.append(pt)
                    if pend is not None:
                        for h in range(2):
                            nc.tensor.matmul(
                                psavs[h][:], lhsT=v2[:, i - 1, vsls[h]],
                                rhs=pend[h][:],
                                start=(i == 1), stop=False)
                    pend = pts
                for h in range(2):
                    nc.tensor.matmul(
                        psavs[h][:], lhsT=v2[:, nkv - 1, vsls[h]],
                        rhs=pend[h][:],
                        start=(nkv == 1), stop=True)
                ots = []
                for h in range(2):
                    rec = wrk2p.tile([65, CH], f32, tag="rec")
                    nc.vector.reciprocal(rec[64:65, :], psavs[h][64:65, :])
                    avs = wrk2p.tile([64, CH], bf16, tag=f"avs{h}",
                                     name=f"avs{h}")
                    nc.scalar.activation(avs[:], psavs[h][0:64, :], AF.Copy)
                    ob = psB.tile([128, 512], f32, tag="psb")
                    nc.tensor.matmul(ob[0:64, 0:CH],
                                     lhsT=onesb[64:65, 0:64],
                                     rhs=rec[64:65, :],
                                     start=True, stop=True)
                    ot = wrk2p.tile([64, CH], bf16, tag=f"ot{h}",
                                    name=f"ot{h}")
                    nc.vector.tensor_mul(ot[:], avs[:], ob[0:64, 0:CH])
                    ots.append(ot)
                # o-projection + resid/8 -> bf16 -> per-chunk AllReduce
                dsm = stagep.tile([128, KT * CH], bf16, tag="dsm",
                                  name=f"dsa{c}")
                res = rarr[c]
                for kt in range(KT):
                    pso = psA.tile([128, CH], f32, tag="psx")
                    nc.tensor.matmul(pso[:], lhsT=wo0[:, kt, :], rhs=ots[0][:],
                                     start=True, stop=False)
                    nc.tensor.matmul(pso[:], lhsT=wo1[:, kt, :], rhs=ots[1][:],
                                     start=False, stop=False)
                    nc.tensor.matmul(pso[:], lhsT=eye8[:],
                                     rhs=res[:, rsl(kt)],
                                     start=False, stop=True)
                    nc.vector.tensor_copy(dsm[:, rsl(kt)], pso[:])
                return start_ar(c, "a", dsm)

            def start_ar(c, tagc, dsm):
                arin = dramp.tile([128, KT * CH], bf16, tag=f"ari{tagc}{c}",
                                  name=f"ari{tagc}{c}")
                arout = dramp.tile([128, KT * CH], bf16, tag=f"aro{tagc}{c}",
                                   name=f"aro{tagc}{c}", addr_space="Shared")
                nc.sync.dma_start(arin[:], dsm[:])
                nc.gpsimd.collective_compute(
                    "AllReduce", ALU.add,
                    ins=[arin.opt()], outs=[arout.opt()],
                    replica_groups=[list(range(NCORES))])
                return arout

            def mlp(c, wg, wu, wd):
                for mt in range(MT_I):
                    msl = slice(mt * 128, (mt + 1) * 128)
                    psg = psA.tile([128, CH], f32, tag="psx")
                    for kt in range(KT):
                        nc.tensor.matmul(
                            psg[:], lhsT=wg[:, kt, msl], rhs=xnorms[c][kt][:],
                            start=(kt == 0), stop=(kt == KT - 1))
                    gts = castp.tile([128, CH], bf16, tag="gts", bufs=2)
                    nc.scalar.activation(gts[:], psg[:], AF.Silu)
                    psu = psB.tile([128, 512], f32, tag="psb")
                    for kt in range(KT):
                        nc.tensor.matmul(
                            psu[:, 0:CH], lhsT=wu[:, kt, msl],
                            rhs=xnorms[c][kt][:],
                            start=(kt == 0), stop=(kt == KT - 1))
                    nc.vector.tensor_mul(hmlps[c][mt][:], gts[:],
                                         psu[:, 0:CH])
                dsm = stagep.tile([128, KT * CH], bf16, tag="dsm",
                                  name=f"dsm{c}")
                res = rarr[c]
                for kt in range(KT):
                    psd = psA.tile([128, CH], f32, tag="psx")
                    for mt in range(MT_I):
                        nc.tensor.matmul(
                            psd[:], lhsT=wd[:, mt, kt * 128:(kt + 1) * 128],
                            rhs=hmlps[c][mt][:],
                            start=(mt == 0), stop=False)
                    nc.tensor.matmul(psd[:], lhsT=eye8[:],
                                     rhs=res[:, rsl(kt)],
                                     start=False, stop=True)
                    if kt % 2 == 0:
                        nc.scalar.activation(dsm[:, rsl(kt)], psd[:], AF.Copy)
                    else:
                        nc.vector.tensor_copy(dsm[:, rsl(kt)], psd[:])
                return dsm

            # ---- conv patch embed (replicated, streamed) + ln_pre ----
            with tc.tile_pool(name="convp", bufs=2) as convp:
                for c in range(NCHUNK):
                    gco = C_LO[c]
                    pch = convp.tile([128, PKT, CH], bf16, tag="pch")
                    nc.sync.dma_start(pch[:], patchesT_d[:, :, gco:gco + CH])
                    cr = new_rarr(c)
                    for kt in range(KT):
                        cwt = convp.tile([128, PKT, 128], bf16, tag="cwt")
                        nc.sync.dma_start(
                            cwt[:], convWT_d[:, :, kt * 128:(kt + 1) * 128])
                        psx = psA.tile([128, CH], f32, tag="psx")
                        for pk in range(PKT):
                            nc.tensor.matmul(
                                psx[:], lhsT=cwt[:, pk, :],
                                rhs=pch[:, pk, :],
                                start=(pk == 0), stop=(pk == PKT - 1))
                        nc.scalar.activation(cr[:, rsl(kt)], psx[:], AF.Copy)
                    ln_pre(c)

            # ---- transformer layers (3-chunk pipelined stream) ----
            ar_m = [None, None, None]
            for l in range(NLAYERS):
                wq = watp.tile([128, KT, 128], bf16, tag="wq")
                wk = watp.tile([128, KT, 128], bf16, tag="wk")
                wv2 = watp.tile([128, KT, 130], bf16, tag="wv2")
                wo0 = watp.tile([64, KT, 128], bf16, tag="wo0")
                wo1 = watp.tile([64, KT, 128], bf16, tag="wo1")
                nc.sync.dma_start(wq[:], wq_d[l])
                nc.sync.dma_start(wk[:], wk_d[l])
                nc.sync.dma_start(wv2[:], wv2_d[l])
                nc.sync.dma_start(wo0[:], wo0_d[l])
                nc.sync.dma_start(wo1[:], wo1_d[l])
                wg = wmlpp.tile([128, KT, 512], bf16, tag="wg")
                wu = wmlpp.tile([128, KT, 512], bf16, tag="wu")
                wd = wmlpp.tile([128, MT_I, HID], bf16, tag="wd")
                nc.sync.dma_start(wg[:], wg_d[l])
                nc.sync.dma_start(wu[:], wu_d[l])
                nc.sync.dma_start(wd[:], wd_d[l])

                # attn norms + q/k for chunks 0,1 first so attention on img0
                # can start while chunk 2's AR from the previous layer lands
                # readbacks are emitted one chunk ahead of their consumer so
                # they never sit behind a later arin-write in the sync queue
                if ar_m[2] is not None:
                    readback(2, ar_m[2])
                for c in (0, 1):
                    mark(f"L{l}nA{c}")
                    norm(c)
                    mark(f"L{l}kq{c}")
                    k_proj(c, wk)
                    q_proj(c, wq)
                mark(f"L{l}v0")
                v_proj(0, wv2)
                mark(f"L{l}nA2")
                norm(2)
                mark(f"L{l}kq2")
                k_proj(2, wk)
                q_proj(2, wq)
                mark(f"L{l}v1")
                v_proj(1, wv2)
                ar_a = []
                for c in range(NCHUNK):
                    mark(f"L{l}at{c}")
                    ar_a.append(attn(c, wo0, wo1))
                    if c >= 1:
                        readback(c - 1, ar_a[c - 1])
                # img0's two mlp partials share ONE 2MB AllReduce (saves a
                # rendezvous); img1 keeps its own
                HW2 = KT * CH
                ari01 = dramp.tile([128, 2 * HW2], bf16, tag="ari01",
                                   name="ari01")
                aro01 = dramp.tile([128, 2 * HW2], bf16, tag="aro01",
                                   name="aro01", addr_space="Shared")
                for c in range(NCHUNK):
                    mark(f"L{l}nF{c}")
                    norm(c)
                    mark(f"L{l}ml{c}")
                    dsm = mlp(c, wg, wu, wd)
                    if c == 0:
                        nc.sync.dma_start(ari01[:, 0:HW2], dsm[:])
                        readback(2, ar_a[2])
                    elif c == 1:
                        nc.sync.dma_start(ari01[:, HW2:], dsm[:])
                        nc.gpsimd.collective_compute(
                            "AllReduce", ALU.add,
                            ins=[ari01.opt()], outs=[aro01.opt()],
                            replica_groups=[list(range(NCORES))])
                    else:
                        ar_m[2] = start_ar(2, "m", dsm)
                        readback(0, aro01[:, 0:HW2])
                        readback(1, aro01[:, HW2:])

            def wout(c):
                lo = C_LO[c]
                for kt in range(KT):
                    fin = wrk1p.tile([128, CH], f32, tag="fin", bufs=2)
                    nc.scalar.activation(fin[:], rarr[c][:, rsl(kt)], AF.Copy)
                    nc.sync.dma_start(out_d[:, kt, lo:lo + CH], fin[:])

            wout(0)
            wout(1)
            readback(2, ar_m[2])
            wout(2)

    mark("end")
    nc.compile()
    import json as _json
    with open("/tmp/phase_marks.json", "w") as f:
        _json.dump(marks, f)
    return nc


# ---------------- host-side prep ----------------

def _im2col(img):
    C, H, W = img.shape
    h, w = H // PATCH, W // PATCH
    p = img.reshape(C, h, PATCH, w, PATCH).transpose(1, 3, 0, 2, 4)
    return p.reshape(h * w, C * PATCH * PATCH)


def _rope_tables():
    freqs = 1.0 / THETA ** (np.arange(0, HD, 2, dtype=np.float64) / HD)
    fh = np.outer(np.arange(MAXSIDE, dtype=np.float64), freqs[::2])
    fw = np.outer(np.arange(MAXSIDE, dtype=np.float64), freqs[1::2])
    pids = np.concatenate([
        (np.arange(h)[:, None] * MAXSIDE + np.arange(w)[None, :]).reshape(-1)
        for h, w in GRIDS])
    inv = np.concatenate([
        np.broadcast_to(fh[:, None, :], (MAXSIDE, MAXSIDE, HD // 4)),
        np.broadcast_to(fw[None, :, :], (MAXSIDE, MAXSIDE, HD // 4))],
        axis=-1).reshape(-1, HD // 2)
    inv = np.concatenate([inv, inv], axis=-1)
    emb = inv[pids]                                   # [S, 64]
    cosT = np.cos(emb).T.astype(np.float32)           # [64, S]
    sinT = np.sin(emb).T.astype(np.float32)
    sinTs = np.concatenate([-sinT[:32], sinT[32:]], axis=0)
    cos2 = np.concatenate([cosT, cosT], axis=0).astype(BF16)
    sin2 = np.concatenate([sinTs, sinTs], axis=0).astype(BF16)
    return np.ascontiguousarray(cos2), np.ascontiguousarray(sin2)


def _ktile(a, last):
    """[L, 1024, last] -> [L, 128, kt, last] (partition-major k-tiles)."""
    L = a.shape[0]
    return np.ascontiguousarray(
        a.reshape(L, -1, 128, last).transpose(0, 2, 1, 3))


def _prep(inputs):
    f32 = np.float32
    patches = np.concatenate([
        _im2col(np.asarray(inputs["img0"], f32)),
        _im2col(np.asarray(inputs["img1"], f32))])          # [S, 768]
    patchesT = np.ascontiguousarray(
        patches.T.reshape(PKT, 128, S).transpose(1, 0, 2)).astype(BF16)
    cw = np.asarray(inputs["conv_w"], f32).reshape(HID, 768)
    convWT = np.ascontiguousarray(
        cw.T.reshape(PKT, 128, HID).transpose(1, 0, 2)).astype(BF16)
    cos2, sin2 = _rope_tables()
    lnw = np.ascontiguousarray(
        np.asarray(inputs["ln_pre_w"], f32).reshape(KT, 128).T)

    anw = np.asarray(inputs["attn_norm_w"], f32)              # [4, 1024]
    fnw = np.asarray(inputs["ffn_norm_w"], f32)
    # fold the norm weights into the following projections (input dim)
    qwT = np.asarray(inputs["q_w"], f32).transpose(0, 2, 1) * anw[:, :, None]
    kwT = np.asarray(inputs["k_w"], f32).transpose(0, 2, 1) * anw[:, :, None]
    vwT = np.asarray(inputs["v_w"], f32).transpose(0, 2, 1) * anw[:, :, None]
    owT = np.asarray(inputs["o_w"], f32).transpose(0, 2, 1)   # [4, d, e]
    gwT = np.asarray(inputs["gate_w"], f32).transpose(0, 2, 1) * fnw[:, :, None]
    uwT = np.asarray(inputs["up_w"], f32).transpose(0, 2, 1) * fnw[:, :, None]
    dwT = np.asarray(inputs["down_w"], f32).transpose(0, 2, 1)  # [4, I, out]

    eye8 = (np.eye(128, dtype=f32) * 0.125).astype(BF16)
    common = dict(patchesT=patchesT, convWT=convWT, cos2=cos2, sin2=sin2,
                  lnw=lnw, eye8=eye8)
    in_maps = []
    for c in range(NCORES):
        esl = slice(c * 128, (c + 1) * 128)
        isl = slice(c * 512, (c + 1) * 512)
        wv = vwT[:, :, esl].astype(BF16)                      # [4, 1024, 128]
        wv2 = np.zeros((NLAYERS, HID, 130), BF16)
        wv2[:, :, 0:64] = wv[:, :, 0:64]
        wv2[:, :, 65:129] = wv[:, :, 64:128]
        wo = owT[:, esl, :]                                   # [4, 128, 1024]
        m = dict(
            wq=_ktile(qwT[:, :, esl].astype(BF16), 128),
            wk=_ktile(kwT[:, :, esl].astype(BF16), 128),
            wv2=_ktile(wv2, 130),
            wo0=np.ascontiguousarray(
                wo[:, 0:64, :].reshape(NLAYERS, 64, KT, 128)).astype(BF16),
            wo1=np.ascontiguousarray(
                wo[:, 64:128, :].reshape(NLAYERS, 64, KT, 128)).astype(BF16),
            wg=_ktile(gwT[:, :, isl].astype(BF16), 512),
            wu=_ktile(uwT[:, :, isl].astype(BF16), 512),
            wd=np.ascontiguousarray(
                dwT[:, isl, :].reshape(NLAYERS, MT_I, 128, HID)
                .transpose(0, 2, 1, 3)).astype(BF16),
            **common)
        in_maps.append(m)
    return in_maps


LAST_RESULTS = None
TRACE = False


def _install_ntff_hook():
    """The RL container's antenv lacks axon_hooks; recreate it so
    trace=True can capture NTFF profiles through the axon terminal."""
    import types
    import antenv

    if hasattr(antenv, "axon_hooks"):
        return
    mod = types.ModuleType("antenv.axon_hooks")
    holder = [None]
    mod.set_axon_ntff_profile_hook = lambda h: holder.__setitem__(0, h)
    mod.get_axon_ntff_profile_hook = lambda: holder[0]
    sys.modules["antenv.axon_hooks"] = mod
    antenv.axon_hooks = mod
    if "/root/.axon_site" not in sys.path:
        sys.path.insert(0, "/root/.axon_site")
    try:
        from trn_agent_boot.trn_boot import _ntff_profile_via_ctypes
        mod.set_axon_ntff_profile_hook(
            _ntff_profile_via_ctypes("/opt/axon/libaxon_pjrt.so"))
    except Exception as e:  # pragma: no cover
        print("ntff hook install failed:", e)


def kernel(**inputs):
    global LAST_RESULTS
    from concourse import bass_utils

    if TRACE:
        _install_ntff_hook()

    if "nc" not in _CACHE:
        _CACHE["nc"] = _build_nc()
    nc = _CACHE["nc"]
    in_maps = _prep(inputs)
    res = bass_utils.run_bass_kernel_spmd(
        nc, in_maps, core_ids=list(range(NCORES)), trace=TRACE)
    LAST_RESULTS = res
    out = res.results[0]["out"]                 # [128, KT, S] f32
    full = out.transpose(1, 0, 2).reshape(HID, S)   # [hid, S]
    return np.ascontiguousarray(full.T[None]).astype(np.float32)
